# revision 10
# baseline (speedup 1.0000x reference)
"""CenterLoss2 Trainium2 kernel.

Problem (reference.py):
    centers_batch = centers[y]                      # [B, D] gather
    loss = 0.01 * mean((centers_batch - x)**2)
    counts = histogram(y)                           # [N]
    delta_i = 0.05 * (centers[y_i] - x_i) / (counts[y_i] + 1e-6)
    new_centers = centers.at[y].add(-delta)

Distribution: centers table is sharded row-wise across the 8 cores
(12500 rows each). (x, y) pairs are routed on the host to the core
owning center y (the "all-to-all" of the sharding hint), sorted by
class. Per unique class c with count n and segment-sum s = sum x_i:

    new_c  = a*c + b*s,   a = 1 - ALPHA*n/(n+eps),  b = ALPHA/(n+eps)
    loss  += n*||c||^2 - 2*c.s   (+ sum ||x_i||^2, added once per sample)

Per-core device work:
    1. bulk DRAM->DRAM copy of the centers shard into the output
    2. indirect-DMA gather of first-occurrence x rows (round 0) and
       duplicate occurrences (rounds 1..R-1, count-desc slot order so
       later rounds are a small slot prefix); summed -> s
    3. indirect-DMA gather of the unique center rows -> c
    4. DVE: square-sums for the loss, new_c = a*c + b*s
    5. indirect-DMA scatter of new_c rows into the output shard
       (slots are unique rows -> no write conflicts; pad slots target a
       trash row appended to the output tensor)
    6. [128,2] loss partials out; host reduces 8 cores' partials.
"""

import math

import numpy as np

import concourse.bacc as bacc
import concourse.bass as bass
import concourse.mybir as mybir
import concourse.tile as tile
from concourse.bass_utils import run_bass_kernel_spmd
from concourse.tile_rust import add_dep_helper

NB_CLASS = 100000
DIM = 256
NCORES = 8
NSH = NB_CLASS // NCORES  # 12500 center rows per core
ALPHA = 0.05
EPS = 1e-6
LOSS_WEIGHT = 0.01
P = 128
F32 = mybir.dt.float32
I32 = mybir.dt.int32

COPY_CHUNKS = 8


def _build_nc(T: int, B_cap: int, round_tiles: list[int], nsh: int = NSH,
              dim: int = DIM, repeat: int = 1):
    """Build the single-core Bass program (same NEFF for all 8 cores).

    T           - number of 128-row unique-class tiles (U_pad = T*128)
    B_cap       - rows in the per-core x_sel table (last row is zeros)
    round_tiles - per duplicate-round j (occurrence j=1..), the number of
                  128-slot tiles covered by that round (count-desc order
                  makes these a prefix of the slot space)
    """
    U_pad = T * P
    RT = sum(round_tiles)
    nc = bacc.Bacc()

    centers_sh = nc.dram_tensor("centers_sh", [nsh, dim], F32, kind="ExternalInput")
    x_sel = nc.dram_tensor("x_sel", [B_cap, dim], F32, kind="ExternalInput")
    pos0 = nc.dram_tensor("pos0", [P, T], I32, kind="ExternalInput")
    clsg = nc.dram_tensor("clsg", [P, T], I32, kind="ExternalInput")
    clss = nc.dram_tensor("clss", [P, T], I32, kind="ExternalInput")
    # coef columns: [0,T) = count, [T,2T) = a, [2T,3T) = b
    coef = nc.dram_tensor("coef", [P, 3 * T], F32, kind="ExternalInput")
    posr = None
    if RT:
        posr = nc.dram_tensor("posr", [P, RT], I32, kind="ExternalInput")

    out_sh = nc.dram_tensor("out_sh", [nsh + 1, dim], F32, kind="ExternalOutput")
    out_stats = nc.dram_tensor("out_stats", [P, 2], F32, kind="ExternalOutput")

    import contextlib

    with tile.TileContext(nc) as tc:
        with (
            tc.tile_pool(name="persist", bufs=1) as pp,
            tc.tile_pool(name="work", bufs=3) as wp,
        ):
            # repeat>1 wraps the body in a HW loop (timing-only builds)
            _loop = contextlib.ExitStack()
            if repeat > 1:
                _loop.enter_context(tc.For_i(0, repeat, 1))
            # ---- bulk copy of the shard (DRAM -> DRAM) ----
            copy_insts = []
            per = math.ceil(nsh / COPY_CHUNKS)
            for i in range(COPY_CHUNKS):
                r0 = i * per
                r1 = min(nsh, r0 + per)
                if r0 >= r1:
                    break
                ci = nc.sync.dma_start(out=out_sh[r0:r1, :], in_=centers_sh[r0:r1, :])
                copy_insts.append(ci)

            # ---- metadata loads ----
            pos0_t = pp.tile([P, T], I32, tag="pos0")
            nc.sync.dma_start(out=pos0_t[:], in_=pos0[:, :])
            clsg_t = pp.tile([P, T], I32, tag="clsg")
            nc.sync.dma_start(out=clsg_t[:], in_=clsg[:, :])
            clss_t = pp.tile([P, T], I32, tag="clss")
            nc.sync.dma_start(out=clss_t[:], in_=clss[:, :])
            coef_t = pp.tile([P, 3 * T], F32, tag="coef")
            nc.sync.dma_start(out=coef_t[:], in_=coef[:, :])
            posr_t = None
            if RT:
                posr_t = pp.tile([P, RT], I32, tag="posr")
                nc.sync.dma_start(out=posr_t[:], in_=posr[:, :])

            # ---- gathers ----
            # slot u = j*128 + p lives at partition p, free cols [j*dim,(j+1)*dim)
            # NOTE: indirect DMA offset tiles must be [P,1] — the HW
            # descriptor mapping for [P,G] offset tiles is swizzled and does
            # not match the flat simulator semantics.
            s_big = pp.tile([P, T * dim], F32, tag="s")
            c_big = pp.tile([P, T * dim], F32, tag="c")
            for t in range(T):
                cslice = slice(t * dim, (t + 1) * dim)
                nc.gpsimd.indirect_dma_start(
                    out=s_big[:, cslice],
                    out_offset=None,
                    in_=x_sel[:, :],
                    in_offset=bass.IndirectOffsetOnAxis(ap=pos0_t[:, t : t + 1], axis=0),
                )
                nc.gpsimd.indirect_dma_start(
                    out=c_big[:, cslice],
                    out_offset=None,
                    in_=centers_sh[:, :],
                    in_offset=bass.IndirectOffsetOnAxis(ap=clsg_t[:, t : t + 1], axis=0),
                )

            # ---- loss square-sum columns ----
            n_xsq = 1 + len(round_tiles)
            xsq_cols = pp.tile([P, n_xsq], F32, tag="xsq")
            sq_big = pp.tile([P, T * dim], F32, tag="sqtmp")
            # ||x||^2 of round-0 rows (must read s_big before round adds)
            nc.vector.tensor_tensor(
                out=sq_big[:], in0=s_big[:], in1=s_big[:], op=mybir.AluOpType.mult
            )
            nc.vector.reduce_sum(
                out=xsq_cols[:, 0:1], in_=sq_big[:], axis=mybir.AxisListType.X
            )

            # ---- duplicate rounds: gather occurrence j and add into s ----
            col = 0
            for j, kt in enumerate(round_tiles):
                g = wp.tile([P, kt * dim], F32, tag="roundg")
                for t in range(kt):
                    nc.gpsimd.indirect_dma_start(
                        out=g[:, t * dim : (t + 1) * dim],
                        out_offset=None,
                        in_=x_sel[:, :],
                        in_offset=bass.IndirectOffsetOnAxis(
                            ap=posr_t[:, col + t : col + t + 1], axis=0
                        ),
                    )
                nc.vector.tensor_tensor(
                    out=sq_big[:, : kt * dim],
                    in0=g[:],
                    in1=g[:],
                    op=mybir.AluOpType.mult,
                )
                nc.vector.reduce_sum(
                    out=xsq_cols[:, 1 + j : 2 + j],
                    in_=sq_big[:, : kt * dim],
                    axis=mybir.AxisListType.X,
                )
                nc.vector.tensor_tensor(
                    out=s_big[:, : kt * dim],
                    in0=s_big[:, : kt * dim],
                    in1=g[:],
                    op=mybir.AluOpType.add,
                )
                col += kt

            # ---- per-tile: loss terms and new_c ----
            lt_cols = pp.tile([P, T], F32, tag="lt")
            newc_big = pp.tile([P, T * dim], F32, tag="newc")
            for t in range(T):
                cs = slice(t * dim, (t + 1) * dim)
                c_t = c_big[:, cs]
                s_t = s_big[:, cs]
                cnt_c = coef_t[:, t : t + 1]
                a_c = coef_t[:, T + t : T + t + 1]
                b_c = coef_t[:, 2 * T + t : 2 * T + t + 1]

                # m = n*c - 2*s ; loss_t = sum(c * m)
                m = wp.tile([P, dim], F32, tag="m")
                nc.vector.tensor_scalar_mul(out=m[:], in0=c_t, scalar1=cnt_c)
                s2 = wp.tile([P, dim], F32, tag="s2")
                nc.vector.tensor_scalar_mul(out=s2[:], in0=s_t, scalar1=2.0)
                nc.vector.tensor_tensor(
                    out=m[:], in0=m[:], in1=s2[:], op=mybir.AluOpType.subtract
                )
                nc.vector.tensor_tensor(
                    out=m[:], in0=m[:], in1=c_t, op=mybir.AluOpType.mult
                )
                nc.vector.reduce_sum(
                    out=lt_cols[:, t : t + 1], in_=m[:], axis=mybir.AxisListType.X
                )

                # new_c = a*c + b*s
                t2 = wp.tile([P, dim], F32, tag="t2")
                nc.vector.tensor_scalar_mul(out=t2[:], in0=s_t, scalar1=b_c)
                nc.vector.tensor_scalar_mul(out=newc_big[:, cs], in0=c_t, scalar1=a_c)
                nc.vector.tensor_tensor(
                    out=newc_big[:, cs],
                    in0=newc_big[:, cs],
                    in1=t2[:],
                    op=mybir.AluOpType.add,
                )

            # ---- scatter new_c rows (after the bulk copy) ----
            for t in range(T):
                scat = nc.gpsimd.indirect_dma_start(
                    out=out_sh[:, :],
                    out_offset=bass.IndirectOffsetOnAxis(ap=clss_t[:, t : t + 1], axis=0),
                    in_=newc_big[:, t * dim : (t + 1) * dim],
                    in_offset=None,
                )
                for ci in copy_insts:
                    add_dep_helper(scat.ins, ci.ins, reason="scatter after bulk copy")

            # ---- stats out ----
            stats = pp.tile([P, 2], F32, tag="stats")
            nc.vector.reduce_sum(
                out=stats[:, 0:1], in_=lt_cols[:], axis=mybir.AxisListType.X
            )
            nc.vector.reduce_sum(
                out=stats[:, 1:2], in_=xsq_cols[:], axis=mybir.AxisListType.X
            )
            nc.sync.dma_start(out=out_stats[:, :], in_=stats[:])
            _loop.close()

    nc.finalize()  # bacc pipeline: wait splitting, reg alloc, nop fusion
    return nc


def _host_prep(x: np.ndarray, y: np.ndarray, centers: np.ndarray):
    """Route/sort pairs per owning core; build index metadata.

    Returns (params, in_maps) where params keys the compiled NEFF shape.
    """
    B = y.shape[0]
    owner = y // NSH

    cores = []
    max_b = 0
    max_u = 0
    max_cnt = 0
    for k in range(NCORES):
        sel = np.nonzero(owner == k)[0]
        y_loc = (y[sel] - k * NSH).astype(np.int64)
        order = np.argsort(y_loc, kind="stable")
        sel = sel[order]
        y_loc = y_loc[order]
        uniq, first_idx, counts = np.unique(
            y_loc, return_index=True, return_counts=True
        )
        # count-desc slot order (stable -> ties by class asc)
        ord2 = np.argsort(-counts, kind="stable")
        uniq, first_idx, counts = uniq[ord2], first_idx[ord2], counts[ord2]
        cores.append((sel, uniq, first_idx, counts))
        max_b = max(max_b, len(sel))
        max_u = max(max_u, len(uniq))
        max_cnt = max(max_cnt, int(counts[0]) if len(counts) else 1)

    T = max(1, math.ceil(max_u / P))
    U_pad = T * P
    B_cap = max_b + 1  # last row stays zeros
    Z = B_cap - 1

    # rounds j = 1..max_cnt-1; slots with count > j are a prefix (count-desc)
    round_tiles = []
    for j in range(1, max_cnt):
        dj = max(int((c[3] > j).sum()) for c in cores)
        round_tiles.append(max(1, math.ceil(dj / P)))

    def to_pt(arr_u, pad_val, dtype, cols):
        full = np.full(cols * P, pad_val, dtype=dtype)
        full[: len(arr_u)] = arr_u
        return np.ascontiguousarray(full.reshape(cols, P).T)

    in_maps = []
    for k in range(NCORES):
        sel, uniq, first_idx, counts = cores[k]
        x_sel = np.zeros((B_cap, DIM), dtype=np.float32)
        if len(sel):
            x_sel[: len(sel)] = x[sel]

        cnt_f = counts.astype(np.float32)
        a_u = (1.0 - ALPHA * cnt_f / (cnt_f + EPS)).astype(np.float32)
        b_u = (ALPHA / (cnt_f + EPS)).astype(np.float32)

        pos0_pt = to_pt(first_idx.astype(np.int32), Z, np.int32, T)
        clsg_pt = to_pt(uniq.astype(np.int32), 0, np.int32, T)
        clss_pt = to_pt(uniq.astype(np.int32), NSH, np.int32, T)
        coef_pt = np.concatenate(
            [
                to_pt(cnt_f, 0.0, np.float32, T),
                to_pt(a_u, 1.0, np.float32, T),
                to_pt(b_u, 0.0, np.float32, T),
            ],
            axis=1,
        )

        in_map = {
            "centers_sh": np.ascontiguousarray(
                centers[k * NSH : (k + 1) * NSH]
            ).astype(np.float32, copy=False),
            "x_sel": x_sel,
            "pos0": pos0_pt,
            "clsg": clsg_pt,
            "clss": clss_pt,
            "coef": np.ascontiguousarray(coef_pt),
        }
        if round_tiles:
            cols = []
            for j in range(1, max_cnt):
                kt = round_tiles[j - 1]
                pj = np.full(kt * P, Z, dtype=np.int32)
                valid = counts > j
                nv = int(valid.sum())
                pj[:nv] = (first_idx[:nv] + j).astype(np.int32)
                cols.append(pj.reshape(kt, P).T)
            in_map["posr"] = np.ascontiguousarray(np.concatenate(cols, axis=1))
        in_maps.append(in_map)

    params = (T, B_cap, tuple(round_tiles))
    return params, in_maps


_NC_CACHE: dict = {}


def kernel(x: np.ndarray, y: np.ndarray, centers: np.ndarray):
    x = np.asarray(x, dtype=np.float32)
    y = np.asarray(y, dtype=np.int32)
    centers = np.asarray(centers, dtype=np.float32)
    B = y.shape[0]

    params, in_maps = _host_prep(x, y, centers)
    if params not in _NC_CACHE:
        _NC_CACHE[params] = _build_nc(*params)
    nc = _NC_CACHE[params]

    res = run_bass_kernel_spmd(nc, in_maps, core_ids=list(range(NCORES)))
    results = res.results

    new_centers = np.empty((NB_CLASS, DIM), dtype=np.float32)
    total = 0.0
    for k in range(NCORES):
        new_centers[k * NSH : (k + 1) * NSH] = results[k]["out_sh"][:NSH]
        st = results[k]["out_stats"].astype(np.float64)
        total += st[:, 0].sum() + st[:, 1].sum()

    loss = np.float32(LOSS_WEIGHT * total / (B * DIM))
    return loss, new_centers


# revision 20
# speedup vs baseline: 2.1606x; 2.1606x over previous
"""CenterLoss2 Trainium2 kernel.

Problem (reference.py):
    centers_batch = centers[y]                      # [B, D] gather
    loss = 0.01 * mean((centers_batch - x)**2)
    counts = histogram(y)                           # [N]
    delta_i = 0.05 * (centers[y_i] - x_i) / (counts[y_i] + 1e-6)
    new_centers = centers.at[y].add(-delta)

Distribution: the centers table is sharded row-wise across the 8 cores
(12500 rows each). (x, y) pairs are routed on the host to the core
owning center y (the "all-to-all" of the sharding hint), sorted by
class. Per unique class c with count n and segment-sum s = sum x_i:

    new_c  = c + delta_c,  delta_c = am1*c + b*s,
    am1 = -ALPHA*n/(n+eps),  b = ALPHA/(n+eps)
    loss += n*||c||^2 - 2*c.s   (+ sum ||x_i||^2, once per sample)

Per-core device work:
    1. bulk copy of the centers shard into the output, split across two
       DGE paths in parallel (SWDGE DRAM->DRAM chunks + HWDGE bounce
       through SBUF) to saturate the ~358 GB/s per-core HBM budget
    2. one dma_gather of first-occurrence x rows (slot u = t*128+p ->
       [p, t, :]) plus one small dma_gather for duplicate occurrences
       (count-desc slot order makes round j a prefix of the slots)
    3. one dma_gather of the unique center rows -> c
    4. DVE: square-sums for the loss, delta = am1*c + b*s
    5. one dma_scatter_add of delta into the copied output shard (slots
       are unique rows; pad slots add 0 to a trash row)
    6. [128,2] loss partials out; host reduces the 8 cores' partials.

All indices are int16 ucode-DMA index tiles: a [16, n/16] block
(entry k at [k%16, k//16]) replicated 8x vertically (one copy per Q7
core), loaded as one [128, cols] tile.
"""

import contextlib
import math

import numpy as np

import concourse.bacc as bacc
import concourse.bass as bass
import concourse.mybir as mybir
import concourse.tile as tile
from concourse.bass_utils import run_bass_kernel_spmd
from concourse.tile_rust import add_dep_helper

NB_CLASS = 100000
DIM = 256
NCORES = 8
NSH = NB_CLASS // NCORES  # 12500 center rows per core
ALPHA = 0.05
EPS = 1e-6
LOSS_WEIGHT = 0.01
P = 128
F32 = mybir.dt.float32
I16 = mybir.dt.int16

D2D_CHUNKS = 10        # SWDGE DRAM->DRAM chunk count (~0.8MB each)
D2D_FRAC_NUM = 5       # fraction of rows on the SWDGE path: 5/8
D2D_FRAC_DEN = 8
BOUNCE_ROWS = 128 * 8  # 1MB HWDGE bounce tiles

ALL_PARTS = frozenset({"copy", "gather", "rounds", "dve", "scatter", "stats"})


def _build_nc(T: int, B_cap: int, round_tiles: tuple, nsh: int = NSH,
              dim: int = DIM, repeat: int = 1, parts: frozenset = ALL_PARTS):
    """Build the single-core Bass program (same NEFF for all 8 cores).

    T           - number of 128-row unique-class tiles (U_pad = T*128)
    B_cap       - rows in the per-core x_sel table (last row is zeros)
    round_tiles - per duplicate-round j, the number of 128-slot tiles
                  covered by that round (a prefix of the slot space)
    """
    RT = sum(round_tiles)
    # idx16 columns: pos0 | cls_gather | cls_scatter | rounds
    c_pos0, c_clsg, c_clss, c_rnd = 0, 8 * T, 16 * T, 24 * T
    COLS = 24 * T + 8 * RT
    nc = bacc.Bacc()

    centers_sh = nc.dram_tensor("centers_sh", [nsh, dim], F32, kind="ExternalInput")
    x_sel = nc.dram_tensor("x_sel", [B_cap, dim], F32, kind="ExternalInput")
    idx16 = nc.dram_tensor("idx16", [P, COLS], I16, kind="ExternalInput")
    # coef columns: [0,T) = count, [T,2T) = am1 = a-1, [2T,3T) = b
    coef = nc.dram_tensor("coef", [P, 3 * T], F32, kind="ExternalInput")

    out_sh = nc.dram_tensor("out_sh", [nsh + 1, dim], F32, kind="ExternalOutput")
    out_stats = nc.dram_tensor("out_stats", [P, 2], F32, kind="ExternalOutput")

    with tile.TileContext(nc) as tc:
        with (
            tc.tile_pool(name="persist", bufs=1) as pp,
            tc.tile_pool(name="work", bufs=4) as wp,
        ):
            # repeat>1 wraps the body in a HW loop (timing-only builds)
            _loop = contextlib.ExitStack()
            if repeat > 1:
                _loop.enter_context(tc.For_i(0, repeat, 1))

            # ---- metadata loads ----
            idx_t = pp.tile([P, COLS], I16, tag="idx")
            nc.sync.dma_start(out=idx_t[:], in_=idx16[:, :])
            coef_t = pp.tile([P, 3 * T], F32, tag="coef")
            nc.sync.dma_start(out=coef_t[:], in_=coef[:, :])

            # ---- bulk copy of the shard, two parallel DGE paths ----
            copy_insts = []
            if "copy" in parts:
                split = (nsh * D2D_FRAC_NUM // D2D_FRAC_DEN) // P * P
                per = math.ceil(split / D2D_CHUNKS)
                for i in range(D2D_CHUNKS):
                    r0, r1 = i * per, min(split, (i + 1) * per)
                    if r0 >= r1:
                        break
                    copy_insts.append(
                        nc.gpsimd.dma_start(out=out_sh[r0:r1, :],
                                            in_=centers_sh[r0:r1, :])
                    )
                n_t = math.ceil((nsh - split) / BOUNCE_ROWS)
                for i in range(n_t):
                    r0 = split + i * BOUNCE_ROWS
                    r1 = min(nsh, r0 + BOUNCE_ROWS)
                    nr = r1 - r0
                    if nr % P == 0:
                        a = nr // P
                        t_ = wp.tile([P, a, dim], F32, tag="bounce")
                        v_in = centers_sh[r0:r1, :].rearrange("(a p) d -> p a d", a=a)
                        v_out = out_sh[r0:r1, :].rearrange("(a p) d -> p a d", a=a)
                        nc.sync.dma_start(out=t_[:], in_=v_in)
                        copy_insts.append(nc.scalar.dma_start(out=v_out, in_=t_[:]))
                    else:
                        for rr in range(r0, r1, P):
                            re = min(nsh, rr + P)
                            t_ = wp.tile([P, dim], F32, tag="bounce_sm")
                            nc.sync.dma_start(out=t_[: re - rr, :],
                                              in_=centers_sh[rr:re, :])
                            copy_insts.append(
                                nc.scalar.dma_start(out=out_sh[rr:re, :],
                                                    in_=t_[: re - rr, :])
                            )

            # ---- gathers (slot u = t*128 + p lives at [p, t, :]) ----
            # ucode DMAs are chunked at 4 tiles (512 indices): the 16KB
            # SWDGE descriptor scratch caps ~512 descriptors/instruction.
            UCH = 4

            def _gather(dst, src_ap, col0, tiles):
                for q0 in range(0, tiles, UCH):
                    q1 = min(tiles, q0 + UCH)
                    n = (q1 - q0) * P
                    nc.gpsimd.dma_gather(
                        out_ap=dst[:, q0:q1, :], in_ap=src_ap,
                        idxs_ap=idx_t[:, col0 + 8 * q0 : col0 + 8 * q1],
                        num_idxs=n, num_idxs_reg=n, elem_size=dim,
                    )

            s_big = pp.tile([P, T, dim], F32, tag="s")
            c_big = pp.tile([P, T, dim], F32, tag="c")
            if "gather" in parts:
                _gather(s_big, x_sel[:, :], c_pos0, T)
                _gather(c_big, centers_sh[:, :], c_clsg, T)

            # ---- loss square-sum columns ----
            n_xsq = 1 + len(round_tiles)
            xsq_cols = pp.tile([P, n_xsq], F32, tag="xsq")
            sq_big = pp.tile([P, T, dim], F32, tag="sqtmp")
            if "dve" in parts:
                # ||x||^2 of round-0 rows (read s_big before round adds)
                nc.vector.tensor_tensor(
                    out=sq_big[:], in0=s_big[:], in1=s_big[:],
                    op=mybir.AluOpType.mult,
                )
                nc.vector.reduce_sum(
                    out=xsq_cols[:, 0:1], in_=sq_big[:], axis=mybir.AxisListType.XY
                )

            # ---- duplicate rounds: one gather, add into s prefix ----
            if RT and "rounds" in parts:
                g_all = pp.tile([P, RT, dim], F32, tag="roundg")
                if "gather" in parts:
                    _gather(g_all, x_sel[:, :], c_rnd, RT)
                if "dve" in parts:
                    off = 0
                    for j, kt in enumerate(round_tiles):
                        g_j = g_all[:, off : off + kt, :]
                        nc.vector.tensor_tensor(
                            out=sq_big[:, :kt, :], in0=g_j, in1=g_j,
                            op=mybir.AluOpType.mult,
                        )
                        nc.vector.reduce_sum(
                            out=xsq_cols[:, 1 + j : 2 + j],
                            in_=sq_big[:, :kt, :], axis=mybir.AxisListType.XY,
                        )
                        nc.vector.tensor_tensor(
                            out=s_big[:, :kt, :], in0=s_big[:, :kt, :], in1=g_j,
                            op=mybir.AluOpType.add,
                        )
                        off += kt

            # ---- per-tile: loss terms and delta = am1*c + b*s ----
            lt_cols = pp.tile([P, T], F32, tag="lt")
            dl_big = pp.tile([P, T, dim], F32, tag="delta")
            for t in range(T if "dve" in parts else 0):
                c_t = c_big[:, t, :]
                s_t = s_big[:, t, :]
                cnt_c = coef_t[:, t : t + 1]
                am1_c = coef_t[:, T + t : T + t + 1]
                b_c = coef_t[:, 2 * T + t : 2 * T + t + 1]

                # m = n*c - 2*s ; loss_t = sum(c * m)
                m = wp.tile([P, dim], F32, tag="m")
                nc.vector.tensor_scalar_mul(out=m[:], in0=c_t, scalar1=cnt_c)
                s2 = wp.tile([P, dim], F32, tag="s2")
                nc.vector.tensor_scalar_mul(out=s2[:], in0=s_t, scalar1=2.0)
                nc.vector.tensor_tensor(
                    out=m[:], in0=m[:], in1=s2[:], op=mybir.AluOpType.subtract
                )
                nc.vector.tensor_tensor(
                    out=m[:], in0=m[:], in1=c_t, op=mybir.AluOpType.mult
                )
                nc.vector.reduce_sum(
                    out=lt_cols[:, t : t + 1], in_=m[:], axis=mybir.AxisListType.X
                )

                # delta = am1*c + b*s
                t2 = wp.tile([P, dim], F32, tag="t2")
                nc.vector.tensor_scalar_mul(out=t2[:], in0=s_t, scalar1=b_c)
                nc.vector.tensor_scalar_mul(out=dl_big[:, t, :], in0=c_t,
                                            scalar1=am1_c)
                nc.vector.tensor_tensor(
                    out=dl_big[:, t, :], in0=dl_big[:, t, :], in1=t2[:],
                    op=mybir.AluOpType.add,
                )

            # ---- scatter-add delta rows (after the bulk copy) ----
            if "scatter" in parts:
                for q0 in range(0, T, UCH):
                    q1 = min(T, q0 + UCH)
                    n = (q1 - q0) * P
                    sa = nc.gpsimd.dma_scatter_add(
                        out_ap=out_sh[:, :], in_ap=dl_big[:, q0:q1, :],
                        idxs_ap=idx_t[:, c_clss + 8 * q0 : c_clss + 8 * q1],
                        num_idxs=n, num_idxs_reg=n, elem_size=dim,
                    )
                    for ci in copy_insts:
                        add_dep_helper(sa.ins, ci.ins,
                                       reason="scatter after bulk copy")

            # ---- stats out ----
            if "stats" in parts:
                stats = pp.tile([P, 2], F32, tag="stats")
                nc.vector.reduce_sum(
                    out=stats[:, 0:1], in_=lt_cols[:], axis=mybir.AxisListType.X
                )
                nc.vector.reduce_sum(
                    out=stats[:, 1:2], in_=xsq_cols[:], axis=mybir.AxisListType.X
                )
                nc.sync.dma_start(out=out_stats[:, :], in_=stats[:])
            _loop.close()

    nc.finalize()  # bacc pipeline: wait splitting, reg alloc, nop fusion
    return nc


def _wrap16(flat: np.ndarray) -> np.ndarray:
    """int16 ucode index layout: [16, n/16] block (k -> [k%16, k//16])
    replicated 8x vertically -> [128, n/16]."""
    blk = flat.astype(np.int16).reshape(-1, 16).T
    return np.tile(blk, (8, 1))


def _host_prep(x: np.ndarray, y: np.ndarray, centers: np.ndarray):
    """Route/sort pairs per owning core; build index metadata."""
    owner = y // NSH

    cores = []
    max_b = 0
    max_u = 0
    max_cnt = 1
    for k in range(NCORES):
        sel = np.nonzero(owner == k)[0]
        y_loc = (y[sel] - k * NSH).astype(np.int64)
        order = np.argsort(y_loc, kind="stable")
        sel = sel[order]
        y_loc = y_loc[order]
        uniq, first_idx, counts = np.unique(
            y_loc, return_index=True, return_counts=True
        )
        # count-desc slot order (stable -> ties by class asc)
        ord2 = np.argsort(-counts, kind="stable")
        uniq, first_idx, counts = uniq[ord2], first_idx[ord2], counts[ord2]
        cores.append((sel, uniq, first_idx, counts))
        max_b = max(max_b, len(sel))
        max_u = max(max_u, len(uniq))
        if len(counts):
            max_cnt = max(max_cnt, int(counts[0]))

    T = max(1, math.ceil(max_u / P))
    U_pad = T * P
    B_cap = max_b + 1  # last row stays zeros
    Z = B_cap - 1
    assert B_cap < 32767 and NSH + 1 < 32767  # int16 index tiles

    # rounds j = 1..max_cnt-1; slots with count > j are a prefix (count-desc)
    round_tiles = []
    for j in range(1, max_cnt):
        dj = max(int((c[3] > j).sum()) for c in cores)
        round_tiles.append(max(1, math.ceil(dj / P)))
    round_tiles = tuple(round_tiles)
    RT = sum(round_tiles)

    def pad_to(arr, n, pad_val, dtype=np.float32):
        full = np.full(n, pad_val, dtype=dtype)
        full[: len(arr)] = arr
        return full

    def to_pt(arr_u, pad_val, cols):  # [128, cols] f32 (coef layout)
        full = pad_to(arr_u, cols * P, pad_val)
        return np.ascontiguousarray(full.reshape(cols, P).T)

    in_maps = []
    for k in range(NCORES):
        sel, uniq, first_idx, counts = cores[k]
        x_sel = np.zeros((B_cap, DIM), dtype=np.float32)
        if len(sel):
            x_sel[: len(sel)] = x[sel]

        cnt_f = counts.astype(np.float32)
        am1_u = (-ALPHA * cnt_f / (cnt_f + EPS)).astype(np.float32)
        b_u = (ALPHA / (cnt_f + EPS)).astype(np.float32)

        idx_cols = [
            _wrap16(pad_to(first_idx, U_pad, Z, np.int64)),
            _wrap16(pad_to(uniq, U_pad, 0, np.int64)),
            _wrap16(pad_to(uniq, U_pad, NSH, np.int64)),
        ]
        for j in range(1, max_cnt):
            kt = round_tiles[j - 1]
            pj = np.full(kt * P, Z, dtype=np.int64)
            valid = counts > j
            nv = int(valid.sum())
            pj[:nv] = first_idx[:nv] + j
            idx_cols.append(_wrap16(pj))
        idx16 = np.ascontiguousarray(np.concatenate(idx_cols, axis=1))

        coef_pt = np.concatenate(
            [to_pt(cnt_f, 0.0, T), to_pt(am1_u, 0.0, T), to_pt(b_u, 0.0, T)],
            axis=1,
        )

        in_maps.append({
            "centers_sh": np.ascontiguousarray(
                centers[k * NSH : (k + 1) * NSH]
            ).astype(np.float32, copy=False),
            "x_sel": x_sel,
            "idx16": idx16,
            "coef": np.ascontiguousarray(coef_pt),
        })

    params = (T, B_cap, round_tiles)
    return params, in_maps


_NC_CACHE: dict = {}


def kernel(x: np.ndarray, y: np.ndarray, centers: np.ndarray):
    x = np.asarray(x, dtype=np.float32)
    y = np.asarray(y, dtype=np.int32)
    centers = np.asarray(centers, dtype=np.float32)
    B = y.shape[0]

    params, in_maps = _host_prep(x, y, centers)
    if params not in _NC_CACHE:
        _NC_CACHE[params] = _build_nc(*params)
    nc = _NC_CACHE[params]

    res = run_bass_kernel_spmd(nc, in_maps, core_ids=list(range(NCORES)))
    results = res.results

    new_centers = np.empty((NB_CLASS, DIM), dtype=np.float32)
    total = 0.0
    for k in range(NCORES):
        new_centers[k * NSH : (k + 1) * NSH] = results[k]["out_sh"][:NSH]
        st = results[k]["out_stats"].astype(np.float64)
        total += st[:, 0].sum() + st[:, 1].sum()

    loss = np.float32(LOSS_WEIGHT * total / (B * DIM))
    return loss, new_centers
